# revision 19
# baseline (speedup 1.0000x reference)
"""Trainium2 Bass kernel for a DGL-style digit-capsule routing layer.

Inputs (full, unsharded):
    x      [256, 8, 1152] f32   -- B, D_IN, N_IN
    weight [1152, 10, 16, 8] f32 -- N_IN, N_OUT, D_OUT, D_IN
Output:
    v      [256, 10, 16] f32

Algorithm (exact refactor of the reference, never materializing u_hat):
    s[b,(j,o)]    = sum_{(i,d)} c[i,j] W[(i,d),(j,o)] x[b,(i,d)]     (matmul over (i,d))
    v             = squash(s)
    b_upd[i,j]    = (1/B) sum_d sum_o W[(i,d),(j,o)] M[(i,d),(j,o)]
      where M[(i,d),(j,o)] = sum_b x[b,(i,d)] v[b,(j,o)]             (matmul over b)

Sharding: input capsules i are split 8 ways (144 per core). Per routing
iteration the only cross-core data is the partial sum of s ([256,160],
carried in fp16): iterations 1-2 use AllReduce; iteration 3 uses
AllToAll + an on-core partition-sum matmul (A2A is ~2x cheaper than
ReduceScatter since the CCE reduce needs two source reads per chunk),
after which each core squashes and emits its own 32-batch-row shard.

Perf notes vs the v2 kernel (71.2us baseline):
  - final iteration: ReduceScatter (9.9us) -> AllToAll (~6us) + one
    tiled-identity matmul that sums the 8 gathered 16-row blocks on the
    idle tensor engine, landing full s for this core's batch slice in
    PSUM fp32 (squash squares it on the scalar engine: one PSUM read).
  - agreement W*M multiplies and o-reductions fused 9->3 ops each; each
    PSUM tile now holds three CONSECUTIVE chunks so the fused vector
    work for tile g starts after g*6+6 of the 18 M-matmuls.
  - Wc computed in 3 fused ops (30 capsule groups each) instead of 9.
  - squash Newton chain reordered to drop one multiply; the b_ch
    bookkeeping copy runs in parallel with the scalar-engine Exp.
"""

import numpy as np

N_CORES = 8
B = 256
NI, NO, DO, DI = 1152, 10, 16, 8
JO = NO * DO            # 160
IL = NI // N_CORES      # 144 capsules per core
ID = IL * DI            # 1152 (i,d) rows per core
NCH = ID // 128         # 9 partition chunks
BL = B // N_CORES       # 32 output batch rows per core
SQRT_MAGIC = 0x1FBD1DF5  # bits(sqrt(x)) ~= (bits(x)>>1) + MAGIC

_STATE = {}


def _register_dve_ops():
    """Register the fused sqrt-Newton custom DVE op (idempotent).

    SQUASH_SQRT_NR: out = (q*y0) * (c0 - c1*(q*y0*y0)) with Src0=q,
    Src1=y0~rsqrt(q) seed, c0=1.5, c1=0.5 -> one-instruction Newton
    polish producing sqrt(q) to ~0.2%.
    """
    import concourse.dve_ops as dops
    from concourse.dve_ops import DveOp, DveOpSpec
    from concourse.dve_spec import Spec, Src0, Src1, C0, C1, lower

    if "SQUASH_SQRT_NR" in dops._SUB_OPCODE_FOR_NAME:
        for op in dops.OPS:
            if op.name == "SQUASH_SQRT_NR":
                return op

    a = Src0 * Src1
    body = a * (C0 - C1 * (a * Src1))

    def _ref(in0, in1, c0, c1, c2):
        aa = in0 * in1
        return aa * (c0 - c1 * (aa * in1))

    op = DveOp("SQUASH_SQRT_NR", Spec(body=body, reference=_ref),
               subdim=False, uops_sha={})
    dops.OPS.append(op)
    dops.CUSTOM_DVE_SPECS[op.name] = op.spec
    dops._SUB_OPCODE_FOR_NAME[op.name] = (
        dops._CUSTOM_DVE_ROW_BASE + len(dops.OPS) - 1)
    for ver in ("v3", "v4"):
        uops = lower(op.spec, ver=ver)
        op.uops_sha[ver] = DveOpSpec(
            name=op.name, opcode=dops.get_dve_sub_opcode(op.name),
            uops=uops, rd1_en=True).sha(ver)
    return op


def _build(repeat=1):
    """Build the bass program. ``repeat`` > 1 duplicates the full routing
    computation (for slope-based HW timing); the output is unchanged."""
    import concourse.bass as bass
    import concourse.bacc as bacc
    import concourse.mybir as mybir
    import concourse.tile as tile

    dt = mybir.dt
    AF = mybir.ActivationFunctionType
    ALU = mybir.AluOpType

    sq_op = _register_dve_ops()
    nc = bacc.Bacc(None, num_devices=N_CORES)

    # Per-core external inputs (pre-sharded/pre-laid-out on host).
    xr = nc.declare_dram_parameter("xr", [128, NCH * B], dt.float16, isOutput=False)
    wt = nc.declare_dram_parameter("wt", [128, NCH * JO], dt.float16, isOutput=False)
    xid = nc.declare_dram_parameter("xid", [128, 2 * ID], dt.float16, isOutput=False)
    amat = nc.declare_dram_parameter("amat", [128, 128], dt.float16, isOutput=False)
    tsum = nc.declare_dram_parameter("tsum", [128, 16], dt.float16, isOutput=False)
    v_out = nc.declare_dram_parameter("v_out", [16, 2 * JO], dt.float32,
                                      isOutput=True)

    # Internal DRAM bounce buffers for the collectives (per repeat x iter).
    # Collective payloads live as [128, 2*JO]: batch half kb sits in the
    # column range [kb*JO, (kb+1)*JO) so each hop is ONE contiguous DMA.
    cc_in_all, cc_out_all = [], []
    for rep in range(repeat):
        cc_in_all.append(
            [nc.dram_tensor(f"cc_in{rep}_{t}", [128, 2 * JO], dt.float16)
             for t in range(3)])
        # AllReduce outputs need Shared addr space; AllToAll must not
        cc_out_all.append(
            [nc.dram_tensor(f"cc_out{rep}_{t}", [128, 2 * JO], dt.float16,
                            **({"addr_space": "Shared"} if t < 2 else {}))
             for t in range(3)])
    groups = [list(range(N_CORES))]

    with tile.TileContext(nc) as tc:
        with tc.tile_pool(name="const", bufs=1) as cpool, \
             tc.tile_pool(name="work", bufs=2) as wpool, \
             tc.tile_pool(name="sq", bufs=2) as qpool, \
             tc.tile_pool(name="psum_s", bufs=2, space="PSUM") as ps_pool, \
             tc.tile_pool(name="psum_m", bufs=1, space="PSUM") as pm_pool, \
             tc.tile_pool(name="psum_v", bufs=1, space="PSUM") as pv_pool, \
             tc.tile_pool(name="psum_b", bufs=1, space="PSUM") as pb_pool:

            # ---- constant/persistent tiles ----
            xr_t = cpool.tile([128, NCH * B], dt.float16, tag="xr")
            wt_t = cpool.tile([128, NCH * JO], dt.float16, tag="wt")
            xid_t = cpool.tile([128, 2 * ID], dt.float16, tag="xid")
            amat_t = cpool.tile([128, 128], dt.float16, tag="amat")
            tsum_t = cpool.tile([128, 16], dt.float16, tag="tsum")
            b_ch = cpool.tile([128, NCH * NO], dt.float32, tag="bch")
            wc_t = cpool.tile([128, NCH * JO], dt.float16, tag="wc")

            nc.sync.dma_start(xr_t[:], xr[:])
            nc.sync.dma_start(wt_t[:], wt[:])
            nc.sync.dma_start(xid_t[:], xid[:])
            nc.sync.dma_start(amat_t[:], amat[:])
            nc.sync.dma_start(tsum_t[:], tsum[:])

            def squash(sf, p, out_dt, tag, ng=NO, in_psum=False):
                """v = s * sqrt(sq)/(1+sq); rsqrt via sqrt-bits trick
                + reciprocal + one Newton step, all on the vector engine.
                ``ng`` capsule groups of DO columns are squashed at once."""
                w = ng * DO
                # fp16 out: all-16-bit tensor_tensor runs the DVE in 2x mode
                t2 = wpool.tile([p, w], dt.float16, tag=f"ssq{tag}")
                if in_psum:
                    # sf*sf would need two PSUM reads; DVE has one PSUM port
                    nc.scalar.activation(t2[:], sf[:], AF.Square)
                else:
                    nc.vector.tensor_mul(t2[:], sf[:], sf[:])
                sq = qpool.tile([p, ng], dt.float32, tag=f"sq{tag}")
                nc.vector.reduce_sum(
                    out=sq[:], in_=t2[:].rearrange("p (j o) -> p j o", j=ng),
                    axis=mybir.AxisListType.X)
                # fac = sqrt(sq)/(1+sq): bits-trick sqrt seed (one fused
                # shift+add), exact reciprocal -> rsqrt seed, one custom-DVE
                # Newton polish, and a one-instruction approx reciprocal of
                # (1+sq) (51 ULP)
                sb = qpool.tile([p, ng], dt.float32, tag=f"sb{tag}")
                nc.vector.tensor_scalar(
                    sb[:].bitcast(dt.uint32), sq[:].bitcast(dt.uint32),
                    1, None, ALU.logical_shift_right)
                nc.vector.tensor_scalar(
                    sb[:].bitcast(dt.uint32), sb[:].bitcast(dt.uint32),
                    SQRT_MAGIC, None, ALU.add)
                y0 = qpool.tile([p, ng], dt.float32, tag=f"y0{tag}")
                nc.vector.reciprocal(y0[:], sb[:])
                g = qpool.tile([p, ng], dt.float32, tag=f"g{tag}")
                nc.vector._custom_dve(sq_op, out=g[:], in0=sq[:], in1=y0[:],
                                      s0=1.5, s1=0.5)
                r1 = qpool.tile([p, ng], dt.float32, tag=f"r1{tag}")
                nc.vector.tensor_scalar_add(r1[:], sq[:], 1.0)
                rd = qpool.tile([p, ng], dt.float32, tag=f"rd{tag}")
                nc.vector.reciprocal_approx_fast(out=rd[:], in_=r1[:])
                f1 = qpool.tile([p, ng], dt.float32, tag=f"f1{tag}")
                nc.vector.tensor_mul(f1[:], g[:], rd[:])
                vt = wpool.tile([p, w], out_dt, tag=f"v{tag}")
                nc.vector.tensor_tensor(
                    out=vt[:].rearrange("p (j o) -> p j o", j=ng),
                    in0=sf[:].rearrange("p (j o) -> p j o", j=ng),
                    in1=f1[:].unsqueeze(2).to_broadcast((p, ng, DO)),
                    op=ALU.mult)
                return vt

            for rep in range(repeat):
              cc_in = cc_in_all[rep]
              cc_out = cc_out_all[rep]
              for t in range(3):
                # ---- s matmul: psum_s[kb] = sum_ci xr[:,ci,kb]^T @ w ----
                rhs_w = wt_t if t == 0 else wc_t
                st = wpool.tile([128, 2 * JO], dt.float16, tag="s_sb")
                for kb in range(2):
                    ps = ps_pool.tile([128, JO], dt.float32, tag="ps")
                    for ci in range(NCH):
                        lhs = xr_t[:, ci * B + kb * 128: ci * B + kb * 128 + 128]
                        rhs = rhs_w[:, ci * JO:(ci + 1) * JO]
                        nc.tensor.matmul(ps[:], lhs, rhs,
                                         start=(ci == 0), stop=(ci == NCH - 1))
                    # PSUM->fp16 convert on the (otherwise idle) scalar engine;
                    # c == 1/10 exactly on iteration 1: fold into the copy
                    stk = st[:, kb * JO:(kb + 1) * JO]
                    if t == 0:
                        nc.scalar.mul(stk, ps[:], 0.1)
                    else:
                        nc.scalar.copy(stk, ps[:])
                # ONE 640B-row DMA for both halves: the collective trigger
                # needs both anyway, and 320B-row descriptors pay the <512B
                # 2x latency penalty. Issued from the scalar queue (which
                # just produced st) to skip a cross-queue semaphore hop.
                nc.scalar.dma_start(cc_in[t][:], st[:])

                if t == 2:
                    # ---- final iter: AllToAll so 16-row block j of this
                    # core's payload lands on core j; the tiled-identity
                    # matmul sums the 8 received blocks -> full s for OUR
                    # 32 batch rows, in PSUM fp32 ----
                    nc.gpsimd.collective_compute(
                        "AllToAll", ALU.bypass, replica_groups=groups,
                        ins=[cc_in[2][:]], outs=[cc_out[2][:]],
                    )
                    aat = wpool.tile([128, 2 * JO], dt.float16, tag="aat")
                    nc.gpsimd.dma_start(aat[:], cc_out[2][:])
                    ps_v = pv_pool.tile([16, 2 * JO], dt.float32, tag="ps_v")
                    nc.tensor.matmul(ps_v[:], tsum_t[:], aat[:],
                                     start=True, stop=True)
                    v3 = squash(ps_v, 16, dt.float32, 3, ng=2 * NO,
                                in_psum=True)
                    nc.sync.dma_start(v_out[:], v3[:])
                    break

                nc.gpsimd.collective_compute(
                    "AllReduce", ALU.add, replica_groups=groups,
                    ins=[cc_in[t][:]], outs=[cc_out[t][:]],
                )

                # ---- squash both batch halves in one chain ----
                # gpsimd just retired the collective wait: issuing the
                # gather-back DMA from its queue skips a semaphore hop
                sf2 = wpool.tile([128, 2 * JO], dt.float16, tag="s_full")
                nc.gpsimd.dma_start(sf2[:], cc_out[t][:])
                v2t = squash(sf2, 128, dt.float16, "b", ng=2 * NO)

                # ---- agreement matmuls: M[(i,d),(j,o)] = sum_b x v ----
                # PSUM tile g holds chunks 3g..3g+2 so the fused W*M multiply
                # + o-reduce for tile g can start after 1/3 of the matmuls.
                qt = wpool.tile([128, NCH * NO], dt.float16, tag="qt")
                pms = []
                for g in range(3):
                    pmg = pm_pool.tile([128, 3 * JO], dt.float32, tag=f"pm{g}")
                    pms.append(pmg)
                for g in range(3):
                    pmg = pms[g]
                    for cl in range(3):
                        ci = 3 * g + cl
                        pm = pmg[:, cl * JO:(cl + 1) * JO]
                        for kb in range(2):
                            lhs = xid_t[:, kb * ID + ci * 128:
                                        kb * ID + ci * 128 + 128]
                            nc.tensor.matmul(
                                pm, lhs, v2t[:, kb * JO:(kb + 1) * JO],
                                start=(kb == 0), stop=(kb == 1))
                    # stage PSUM->fp16 on the idle scalar engine so the W*M
                    # multiply runs all-16-bit (DVE 2x mode, no PSUM port)
                    pmh = wpool.tile([128, 3 * JO], dt.float16, tag=f"pmh{g}")
                    nc.scalar.copy(pmh[:], pmg[:])
                    pt = wpool.tile([128, 3 * JO], dt.float16, tag=f"pt{g}")
                    nc.vector.tensor_mul(
                        pt[:], pmh[:], wt_t[:, 3 * g * JO:(3 * g + 3) * JO])
                    with nc.allow_low_precision("o-sum, fp16"):
                        nc.vector.reduce_sum(
                            out=qt[:, g * 3 * NO:(g + 1) * 3 * NO],
                            in_=pt[:].rearrange("p (j o) -> p j o", j=3 * NO),
                            axis=mybir.AxisListType.X)

                # ---- b_upd: one matmul with constant block-diag A sums d ----
                pb = pb_pool.tile([128, NCH * NO], dt.float32, tag="pb")
                nc.tensor.matmul(pb[:], amat_t[:], qt[:], start=True, stop=True)

                # ---- softmax over j (free dim within each chunk) ----
                e_ch = wpool.tile([128, NCH * NO], dt.float32, tag="ech")
                if t == 0:
                    nc.scalar.activation(e_ch[:], pb[:], AF.Exp)
                else:
                    nc.vector.tensor_add(b_ch[:], b_ch[:], pb[:])
                    nc.scalar.activation(e_ch[:], b_ch[:], AF.Exp)
                z_ch = wpool.tile([128, NCH], dt.float32, tag="zch")
                nc.vector.reduce_sum(
                    out=z_ch[:], in_=e_ch[:].rearrange("p (c j) -> p c j", c=NCH),
                    axis=mybir.AxisListType.X)
                r_ch = wpool.tile([128, NCH], dt.float32, tag="rch")
                nc.vector.reciprocal_approx_fast(out=r_ch[:], in_=z_ch[:])
                c_ch = wpool.tile([128, NCH * NO], dt.float32, tag="cch")
                nc.vector.tensor_tensor(
                    out=c_ch[:].rearrange("p (c j) -> p c j", c=NCH),
                    in0=e_ch[:].rearrange("p (c j) -> p c j", c=NCH),
                    in1=r_ch[:].unsqueeze(2).to_broadcast((128, NCH, NO)),
                    op=ALU.mult)

                # ---- Wc = Wt * c (fp16), 3 chunks per op so the first
                # s-matmul third can start after one op ----
                for g in range(3):
                    nc.vector.tensor_tensor(
                        out=wc_t[:, g * 3 * JO:(g + 1) * 3 * JO]
                            .rearrange("p (j o) -> p j o", j=3 * NO),
                        in0=wt_t[:, g * 3 * JO:(g + 1) * 3 * JO]
                            .rearrange("p (j o) -> p j o", j=3 * NO),
                        in1=c_ch[:, g * 3 * NO:(g + 1) * 3 * NO]
                            .unsqueeze(2).to_broadcast((128, 3 * NO, DO)),
                        op=ALU.mult)

                if t == 0:
                    # b_ch bookkeeping for t=1 -- emitted last so it never
                    # delays the z-reduce/Wc chain (vector queue is in-order)
                    nc.vector.tensor_copy(b_ch[:], pb[:])

    return nc


def _get_runner():
    if "runner" in _STATE:
        return _STATE["runner"]

    import jax
    import numpy as np
    from concourse import bass2jax
    from concourse.bass2jax import (
        _bass_exec_p, install_neuronx_cc_hook, partition_id_tensor)
    from jax.experimental.shard_map import shard_map
    from jax.sharding import Mesh, PartitionSpec
    import concourse.mybir as mybir

    nc = _build()
    if not nc.is_finalized():
        nc.finalize()
    install_neuronx_cc_hook()

    partition_name = nc.partition_id_tensor.name if nc.partition_id_tensor else None
    in_names, out_names, out_avals, zero_outs = [], [], [], []
    for alloc in nc.m.functions[0].allocations:
        if not isinstance(alloc, mybir.MemoryLocationSet):
            continue
        name = alloc.memorylocations[0].name
        if alloc.kind == "ExternalInput":
            if name != partition_name:
                in_names.append(name)
        elif alloc.kind == "ExternalOutput":
            out_names.append(name)
            shape = tuple(alloc.tensor_shape)
            dtype = mybir.dt.np(alloc.dtype)
            out_avals.append(jax.core.ShapedArray(shape, dtype))
            zero_outs.append(np.zeros(shape, dtype))
    n_params = len(in_names)
    n_outs = len(out_avals)
    all_names = in_names + out_names
    if partition_name is not None:
        all_names = all_names + [partition_name]

    def _body(*args):
        operands = list(args)
        if partition_name is not None:
            operands.append(partition_id_tensor())
        outs = _bass_exec_p.bind(
            *operands,
            out_avals=tuple(out_avals),
            in_names=tuple(all_names),
            out_names=tuple(out_names),
            lowering_input_output_aliases=(),
            sim_require_finite=True,
            sim_require_nnan=True,
            nc=nc,
        )
        return tuple(outs)

    devices = jax.devices()[:N_CORES]
    assert len(devices) == N_CORES, f"need {N_CORES} cores, have {len(devices)}"
    mesh = Mesh(np.asarray(devices), ("core",))
    donate = tuple(range(n_params, n_params + n_outs))
    sharded = jax.jit(
        shard_map(_body, mesh=mesh,
                  in_specs=(PartitionSpec("core"),) * (n_params + n_outs),
                  out_specs=(PartitionSpec("core"),) * n_outs,
                  check_rep=False),
        donate_argnums=donate, keep_unused=True)

    runner = (sharded, in_names, out_names, [z.shape for z in zero_outs],
              [z.dtype for z in zero_outs])
    _STATE["runner"] = runner
    _STATE["nc"] = nc
    return runner


def _prep_core_inputs(x, weight, k):
    """Host-side slicing/layout for core k (i-shard of 144 capsules)."""
    i0, i1 = k * IL, (k + 1) * IL
    xs = np.ascontiguousarray(x[:, :, i0:i1])          # [256, 8, 144]
    ws = np.ascontiguousarray(weight[i0:i1])           # [144, 10, 16, 8]

    # [(i,d), b] i-major rows, then partition-chunked to [128, 9*256]
    xr = xs.transpose(2, 1, 0).reshape(ID, B)
    xr_ch = xr.reshape(NCH, 128, B).transpose(1, 0, 2).reshape(128, NCH * B)
    # [b, (i,d)] i-major cols, b-chunked to [128, 2*1152]
    xid = xs.transpose(0, 2, 1).reshape(B, IL * DI)
    xid_ch = np.concatenate([xid[0:128], xid[128:256]], axis=1)
    # [(i,d), (j,o)] -> chunked [128, 9*160]
    wt = ws.transpose(0, 3, 1, 2).reshape(ID, JO)
    wt_ch = wt.reshape(NCH, 128, JO).transpose(1, 0, 2).reshape(128, NCH * JO)

    return {
        "xr": xr_ch.astype(np.float16),
        "wt": wt_ch.astype(np.float16),
        "xid": xid_ch.astype(np.float16),
        "amat": _STATE["amat"],
        "tsum": _STATE["tsum"],
    }


def _selectors():
    if "amat" not in _STATE:
        # block-diagonal (i,d)->(i,d') d-sum matrix with the 1/B mean folded in
        p = np.arange(128)
        _STATE["amat"] = ((p[:, None] // DI == p[None, :] // DI)
                          .astype(np.float32) / B).astype(np.float16)
        # [128,16] tiled identity: sums the 8 16-row A2A blocks via matmul
        _STATE["tsum"] = np.tile(np.eye(16, dtype=np.float16), (8, 1))


def kernel(x, weight):
    x = np.asarray(x, dtype=np.float32)
    weight = np.asarray(weight, dtype=np.float32)
    _selectors()
    sharded, in_names, out_names, out_shapes, out_dtypes = _get_runner()

    per_core = [_prep_core_inputs(x, weight, k) for k in range(N_CORES)]
    concat_in = [
        np.concatenate([per_core[c][nm] for c in range(N_CORES)], axis=0)
        for nm in in_names
    ]
    concat_zero = [
        np.zeros((N_CORES * s[0],) + tuple(s[1:]), d)
        for s, d in zip(out_shapes, out_dtypes)
    ]
    outs = sharded(*concat_in, *concat_zero)
    v = np.asarray(outs[out_names.index("v_out")])   # [8*16, 320]
    full = np.empty((B, JO), np.float32)
    for k in range(N_CORES):
        vk = v[k * 16:(k + 1) * 16]
        full[k * 16:k * 16 + 16] = vk[:, :JO]
        full[128 + k * 16:128 + k * 16 + 16] = vk[:, JO:]
    return full.reshape(B, NO, DO)


# revision 20
# speedup vs baseline: 1.0327x; 1.0327x over previous
"""Trainium2 Bass kernel for a DGL-style digit-capsule routing layer.

Inputs (full, unsharded):
    x      [256, 8, 1152] f32   -- B, D_IN, N_IN
    weight [1152, 10, 16, 8] f32 -- N_IN, N_OUT, D_OUT, D_IN
Output:
    v      [256, 10, 16] f32

Algorithm (exact refactor of the reference, never materializing u_hat):
    s[b,(j,o)]    = sum_{(i,d)} c[i,j] W[(i,d),(j,o)] x[b,(i,d)]     (matmul over (i,d))
    v             = squash(s)
    b_upd[i,j]    = (1/B) sum_d sum_o W[(i,d),(j,o)] M[(i,d),(j,o)]
      where M[(i,d),(j,o)] = sum_b x[b,(i,d)] v[b,(j,o)]             (matmul over b)

Sharding: input capsules i are split 8 ways (144 per core). Per routing
iteration the only cross-core data is the partial sum of s ([256,160],
carried in fp16): iterations 1-2 use AllReduce; iteration 3 uses
AllToAll + an on-core partition-sum matmul (A2A is ~2x cheaper than
ReduceScatter since the CCE reduce needs two source reads per chunk),
after which each core squashes and emits its own 32-batch-row shard.

Perf notes vs the v2 kernel (71.2us baseline):
  - final iteration: ReduceScatter (9.9us) -> AllToAll (~6us) + one
    tiled-identity matmul that sums the 8 gathered 16-row blocks on the
    idle tensor engine, landing full s for this core's batch slice in
    PSUM fp32 (squash squares it on the scalar engine: one PSUM read).
  - agreement W*M multiplies and o-reductions fused 9->3 ops each; each
    PSUM tile now holds three CONSECUTIVE chunks so the fused vector
    work for tile g starts after g*6+6 of the 18 M-matmuls.
  - Wc computed in 3 fused ops (30 capsule groups each) instead of 9.
  - squash Newton chain reordered to drop one multiply; the b_ch
    bookkeeping copy runs in parallel with the scalar-engine Exp.
"""

import numpy as np

N_CORES = 8
B = 256
NI, NO, DO, DI = 1152, 10, 16, 8
JO = NO * DO            # 160
IL = NI // N_CORES      # 144 capsules per core
ID = IL * DI            # 1152 (i,d) rows per core
NCH = ID // 128         # 9 partition chunks
BL = B // N_CORES       # 32 output batch rows per core
SQRT_MAGIC = 0x1FBD1DF5  # bits(sqrt(x)) ~= (bits(x)>>1) + MAGIC

_STATE = {}


def _register_dve_ops():
    """Register the fused sqrt-Newton custom DVE op (idempotent).

    SQUASH_SQRT_NR: out = (q*y0) * (c0 - c1*(q*y0*y0)) with Src0=q,
    Src1=y0~rsqrt(q) seed, c0=1.5, c1=0.5 -> one-instruction Newton
    polish producing sqrt(q) to ~0.2%.
    """
    import concourse.dve_ops as dops
    from concourse.dve_ops import DveOp, DveOpSpec
    from concourse.dve_spec import Spec, Src0, Src1, C0, C1, lower

    if "SQUASH_SQRT_NR" in dops._SUB_OPCODE_FOR_NAME:
        for op in dops.OPS:
            if op.name == "SQUASH_SQRT_NR":
                return op

    a = Src0 * Src1
    body = a * (C0 - C1 * (a * Src1))

    def _ref(in0, in1, c0, c1, c2):
        aa = in0 * in1
        return aa * (c0 - c1 * (aa * in1))

    op = DveOp("SQUASH_SQRT_NR", Spec(body=body, reference=_ref),
               subdim=False, uops_sha={})
    dops.OPS.append(op)
    dops.CUSTOM_DVE_SPECS[op.name] = op.spec
    dops._SUB_OPCODE_FOR_NAME[op.name] = (
        dops._CUSTOM_DVE_ROW_BASE + len(dops.OPS) - 1)
    for ver in ("v3", "v4"):
        uops = lower(op.spec, ver=ver)
        op.uops_sha[ver] = DveOpSpec(
            name=op.name, opcode=dops.get_dve_sub_opcode(op.name),
            uops=uops, rd1_en=True).sha(ver)
    return op


def _build(repeat=1):
    """Build the bass program. ``repeat`` > 1 duplicates the full routing
    computation (for slope-based HW timing); the output is unchanged."""
    import concourse.bass as bass
    import concourse.bacc as bacc
    import concourse.mybir as mybir
    import concourse.tile as tile

    dt = mybir.dt
    AF = mybir.ActivationFunctionType
    ALU = mybir.AluOpType

    sq_op = _register_dve_ops()
    nc = bacc.Bacc(None, num_devices=N_CORES)

    # Per-core external inputs (pre-sharded/pre-laid-out on host).
    xr = nc.declare_dram_parameter("xr", [128, NCH * B], dt.float16, isOutput=False)
    wt = nc.declare_dram_parameter("wt", [128, NCH * JO], dt.float16, isOutput=False)
    xid = nc.declare_dram_parameter("xid", [128, 2 * ID], dt.float16, isOutput=False)
    amat = nc.declare_dram_parameter("amat", [128, 128], dt.float16, isOutput=False)
    tsum = nc.declare_dram_parameter("tsum", [128, 16], dt.float16, isOutput=False)
    v_out = nc.declare_dram_parameter("v_out", [16, 2 * JO], dt.float32,
                                      isOutput=True)

    # Internal DRAM bounce buffers for the collectives (per repeat x iter).
    # Collective payloads live as [128, 2*JO]: batch half kb sits in the
    # column range [kb*JO, (kb+1)*JO) so each hop is ONE contiguous DMA.
    cc_in_all, cc_out_all = [], []
    for rep in range(repeat):
        cc_in_all.append(
            [nc.dram_tensor(f"cc_in{rep}_{t}", [128, 2 * JO], dt.float16)
             for t in range(3)])
        # AllReduce outputs need Shared addr space; AllToAll must not
        cc_out_all.append(
            [nc.dram_tensor(f"cc_out{rep}_{t}", [128, 2 * JO], dt.float16,
                            **({"addr_space": "Shared"} if t < 2 else {}))
             for t in range(3)])
    groups = [list(range(N_CORES))]

    with tile.TileContext(nc) as tc:
        with tc.tile_pool(name="const", bufs=1) as cpool, \
             tc.tile_pool(name="work", bufs=2) as wpool, \
             tc.tile_pool(name="sq", bufs=2) as qpool, \
             tc.tile_pool(name="psum_s", bufs=2, space="PSUM") as ps_pool, \
             tc.tile_pool(name="psum_m", bufs=1, space="PSUM") as pm_pool, \
             tc.tile_pool(name="psum_v", bufs=1, space="PSUM") as pv_pool, \
             tc.tile_pool(name="psum_b", bufs=1, space="PSUM") as pb_pool:

            # ---- constant/persistent tiles ----
            xr_t = cpool.tile([128, NCH * B], dt.float16, tag="xr")
            wt_t = cpool.tile([128, NCH * JO], dt.float16, tag="wt")
            xid_t = cpool.tile([128, 2 * ID], dt.float16, tag="xid")
            amat_t = cpool.tile([128, 128], dt.float16, tag="amat")
            tsum_t = cpool.tile([128, 16], dt.float16, tag="tsum")
            b_ch = cpool.tile([128, NCH * NO], dt.float32, tag="bch")
            wc_t = cpool.tile([128, NCH * JO], dt.float16, tag="wc")

            nc.sync.dma_start(xr_t[:], xr[:])
            nc.sync.dma_start(wt_t[:], wt[:])
            nc.sync.dma_start(xid_t[:], xid[:])
            nc.sync.dma_start(amat_t[:], amat[:])
            nc.sync.dma_start(tsum_t[:], tsum[:])

            def squash(sf, p, out_dt, tag, ng=NO, in_psum=False):
                """v = s * sqrt(sq)/(1+sq); rsqrt via sqrt-bits trick
                + reciprocal + one Newton step, all on the vector engine.
                ``ng`` capsule groups of DO columns are squashed at once."""
                w = ng * DO
                # fp16 out: all-16-bit tensor_tensor runs the DVE in 2x mode
                t2 = wpool.tile([p, w], dt.float16, tag=f"ssq{tag}")
                if in_psum:
                    # sf*sf would need two PSUM reads; DVE has one PSUM port
                    nc.scalar.activation(t2[:], sf[:], AF.Square)
                else:
                    nc.vector.tensor_mul(t2[:], sf[:], sf[:])
                sq = qpool.tile([p, ng], dt.float32, tag=f"sq{tag}")
                nc.vector.reduce_sum(
                    out=sq[:], in_=t2[:].rearrange("p (j o) -> p j o", j=ng),
                    axis=mybir.AxisListType.X)
                # fac = sqrt(sq)/(1+sq): bits-trick sqrt seed (one fused
                # shift+add), exact reciprocal -> rsqrt seed, one custom-DVE
                # Newton polish, and a one-instruction approx reciprocal of
                # (1+sq) (51 ULP)
                sb = qpool.tile([p, ng], dt.float32, tag=f"sb{tag}")
                nc.vector.tensor_scalar(
                    sb[:].bitcast(dt.uint32), sq[:].bitcast(dt.uint32),
                    1, None, ALU.logical_shift_right)
                nc.vector.tensor_scalar(
                    sb[:].bitcast(dt.uint32), sb[:].bitcast(dt.uint32),
                    SQRT_MAGIC, None, ALU.add)
                y0 = qpool.tile([p, ng], dt.float32, tag=f"y0{tag}")
                nc.vector.reciprocal(y0[:], sb[:])
                g = qpool.tile([p, ng], dt.float32, tag=f"g{tag}")
                nc.vector._custom_dve(sq_op, out=g[:], in0=sq[:], in1=y0[:],
                                      s0=1.5, s1=0.5)
                r1 = qpool.tile([p, ng], dt.float32, tag=f"r1{tag}")
                nc.vector.tensor_scalar_add(r1[:], sq[:], 1.0)
                rd = qpool.tile([p, ng], dt.float32, tag=f"rd{tag}")
                nc.vector.reciprocal_approx_fast(out=rd[:], in_=r1[:])
                f1 = qpool.tile([p, ng], dt.float32, tag=f"f1{tag}")
                nc.vector.tensor_mul(f1[:], g[:], rd[:])
                vt = wpool.tile([p, w], out_dt, tag=f"v{tag}")
                nc.vector.tensor_tensor(
                    out=vt[:].rearrange("p (j o) -> p j o", j=ng),
                    in0=sf[:].rearrange("p (j o) -> p j o", j=ng),
                    in1=f1[:].unsqueeze(2).to_broadcast((p, ng, DO)),
                    op=ALU.mult)
                return vt

            for rep in range(repeat):
              cc_in = cc_in_all[rep]
              cc_out = cc_out_all[rep]
              for t in range(3):
                # ---- s matmul: psum_s[kb] = sum_ci xr[:,ci,kb]^T @ w ----
                rhs_w = wt_t if t == 0 else wc_t
                st = wpool.tile([128, 2 * JO], dt.float16, tag="s_sb")
                for kb in range(2):
                    ps = ps_pool.tile([128, JO], dt.float32, tag="ps")
                    for ci in range(NCH):
                        lhs = xr_t[:, ci * B + kb * 128: ci * B + kb * 128 + 128]
                        rhs = rhs_w[:, ci * JO:(ci + 1) * JO]
                        nc.tensor.matmul(ps[:], lhs, rhs,
                                         start=(ci == 0), stop=(ci == NCH - 1))
                    # PSUM->fp16 convert on the (otherwise idle) scalar engine;
                    # c == 1/10 exactly on iteration 1: fold into the copy
                    stk = st[:, kb * JO:(kb + 1) * JO]
                    if t == 0:
                        nc.scalar.mul(stk, ps[:], 0.1)
                    else:
                        nc.scalar.copy(stk, ps[:])
                # ONE 640B-row DMA for both halves: the collective trigger
                # needs both anyway, and 320B-row descriptors pay the <512B
                # 2x latency penalty
                nc.sync.dma_start(cc_in[t][:], st[:])

                if t == 2:
                    # ---- final iter: AllToAll so 16-row block j of this
                    # core's payload lands on core j; the tiled-identity
                    # matmul sums the 8 received blocks -> full s for OUR
                    # 32 batch rows, in PSUM fp32 ----
                    nc.gpsimd.collective_compute(
                        "AllToAll", ALU.bypass, replica_groups=groups,
                        ins=[cc_in[2][:]], outs=[cc_out[2][:]],
                    )
                    aat = wpool.tile([128, 2 * JO], dt.float16, tag="aat")
                    nc.sync.dma_start(aat[:], cc_out[2][:])
                    ps_v = pv_pool.tile([16, 2 * JO], dt.float32, tag="ps_v")
                    nc.tensor.matmul(ps_v[:], tsum_t[:], aat[:],
                                     start=True, stop=True)
                    v3 = squash(ps_v, 16, dt.float32, 3, ng=2 * NO,
                                in_psum=True)
                    nc.sync.dma_start(v_out[:], v3[:])
                    break

                nc.gpsimd.collective_compute(
                    "AllReduce", ALU.add, replica_groups=groups,
                    ins=[cc_in[t][:]], outs=[cc_out[t][:]],
                )

                # ---- squash both batch halves in one chain ----
                sf2 = wpool.tile([128, 2 * JO], dt.float16, tag="s_full")
                nc.sync.dma_start(sf2[:], cc_out[t][:])
                v2t = squash(sf2, 128, dt.float16, "b", ng=2 * NO)

                # ---- agreement matmuls: M[(i,d),(j,o)] = sum_b x v ----
                # PSUM tile g holds chunks 3g..3g+2 so the fused W*M multiply
                # + o-reduce for tile g can start after 1/3 of the matmuls.
                qt = wpool.tile([128, NCH * NO], dt.float16, tag="qt")
                pms = []
                for g in range(3):
                    pmg = pm_pool.tile([128, 3 * JO], dt.float32, tag=f"pm{g}")
                    pms.append(pmg)
                for g in range(3):
                    pmg = pms[g]
                    for cl in range(3):
                        ci = 3 * g + cl
                        pm = pmg[:, cl * JO:(cl + 1) * JO]
                        for kb in range(2):
                            lhs = xid_t[:, kb * ID + ci * 128:
                                        kb * ID + ci * 128 + 128]
                            nc.tensor.matmul(
                                pm, lhs, v2t[:, kb * JO:(kb + 1) * JO],
                                start=(kb == 0), stop=(kb == 1))
                    # stage PSUM->fp16 on the idle scalar engine so the W*M
                    # multiply runs all-16-bit (DVE 2x mode, no PSUM port)
                    pmh = wpool.tile([128, 3 * JO], dt.float16, tag=f"pmh{g}")
                    nc.scalar.copy(pmh[:], pmg[:])
                    pt = wpool.tile([128, 3 * JO], dt.float16, tag=f"pt{g}")
                    nc.vector.tensor_mul(
                        pt[:], pmh[:], wt_t[:, 3 * g * JO:(3 * g + 3) * JO])
                    with nc.allow_low_precision("o-sum, fp16"):
                        nc.vector.reduce_sum(
                            out=qt[:, g * 3 * NO:(g + 1) * 3 * NO],
                            in_=pt[:].rearrange("p (j o) -> p j o", j=3 * NO),
                            axis=mybir.AxisListType.X)

                # ---- b_upd: one matmul with constant block-diag A sums d ----
                pb = pb_pool.tile([128, NCH * NO], dt.float32, tag="pb")
                nc.tensor.matmul(pb[:], amat_t[:], qt[:], start=True, stop=True)

                # ---- softmax over j (free dim within each chunk) ----
                e_ch = wpool.tile([128, NCH * NO], dt.float32, tag="ech")
                if t == 0:
                    nc.scalar.activation(e_ch[:], pb[:], AF.Exp)
                else:
                    nc.vector.tensor_add(b_ch[:], b_ch[:], pb[:])
                    nc.scalar.activation(e_ch[:], b_ch[:], AF.Exp)
                z_ch = wpool.tile([128, NCH], dt.float32, tag="zch")
                nc.vector.reduce_sum(
                    out=z_ch[:], in_=e_ch[:].rearrange("p (c j) -> p c j", c=NCH),
                    axis=mybir.AxisListType.X)
                r_ch = wpool.tile([128, NCH], dt.float32, tag="rch")
                nc.vector.reciprocal_approx_fast(out=r_ch[:], in_=z_ch[:])
                c_ch = wpool.tile([128, NCH * NO], dt.float32, tag="cch")
                nc.vector.tensor_tensor(
                    out=c_ch[:].rearrange("p (c j) -> p c j", c=NCH),
                    in0=e_ch[:].rearrange("p (c j) -> p c j", c=NCH),
                    in1=r_ch[:].unsqueeze(2).to_broadcast((128, NCH, NO)),
                    op=ALU.mult)

                # ---- Wc = Wt * c (fp16), 3 chunks per op so the first
                # s-matmul third can start after one op ----
                for g in range(3):
                    nc.vector.tensor_tensor(
                        out=wc_t[:, g * 3 * JO:(g + 1) * 3 * JO]
                            .rearrange("p (j o) -> p j o", j=3 * NO),
                        in0=wt_t[:, g * 3 * JO:(g + 1) * 3 * JO]
                            .rearrange("p (j o) -> p j o", j=3 * NO),
                        in1=c_ch[:, g * 3 * NO:(g + 1) * 3 * NO]
                            .unsqueeze(2).to_broadcast((128, 3 * NO, DO)),
                        op=ALU.mult)

                if t == 0:
                    # b_ch bookkeeping for t=1 -- emitted last so it never
                    # delays the z-reduce/Wc chain (vector queue is in-order)
                    nc.vector.tensor_copy(b_ch[:], pb[:])

    return nc


def _get_runner():
    if "runner" in _STATE:
        return _STATE["runner"]

    import jax
    import numpy as np
    from concourse import bass2jax
    from concourse.bass2jax import (
        _bass_exec_p, install_neuronx_cc_hook, partition_id_tensor)
    from jax.experimental.shard_map import shard_map
    from jax.sharding import Mesh, PartitionSpec
    import concourse.mybir as mybir

    nc = _build()
    if not nc.is_finalized():
        nc.finalize()
    install_neuronx_cc_hook()

    partition_name = nc.partition_id_tensor.name if nc.partition_id_tensor else None
    in_names, out_names, out_avals, zero_outs = [], [], [], []
    for alloc in nc.m.functions[0].allocations:
        if not isinstance(alloc, mybir.MemoryLocationSet):
            continue
        name = alloc.memorylocations[0].name
        if alloc.kind == "ExternalInput":
            if name != partition_name:
                in_names.append(name)
        elif alloc.kind == "ExternalOutput":
            out_names.append(name)
            shape = tuple(alloc.tensor_shape)
            dtype = mybir.dt.np(alloc.dtype)
            out_avals.append(jax.core.ShapedArray(shape, dtype))
            zero_outs.append(np.zeros(shape, dtype))
    n_params = len(in_names)
    n_outs = len(out_avals)
    all_names = in_names + out_names
    if partition_name is not None:
        all_names = all_names + [partition_name]

    def _body(*args):
        operands = list(args)
        if partition_name is not None:
            operands.append(partition_id_tensor())
        outs = _bass_exec_p.bind(
            *operands,
            out_avals=tuple(out_avals),
            in_names=tuple(all_names),
            out_names=tuple(out_names),
            lowering_input_output_aliases=(),
            sim_require_finite=True,
            sim_require_nnan=True,
            nc=nc,
        )
        return tuple(outs)

    devices = jax.devices()[:N_CORES]
    assert len(devices) == N_CORES, f"need {N_CORES} cores, have {len(devices)}"
    mesh = Mesh(np.asarray(devices), ("core",))
    donate = tuple(range(n_params, n_params + n_outs))
    sharded = jax.jit(
        shard_map(_body, mesh=mesh,
                  in_specs=(PartitionSpec("core"),) * (n_params + n_outs),
                  out_specs=(PartitionSpec("core"),) * n_outs,
                  check_rep=False),
        donate_argnums=donate, keep_unused=True)

    runner = (sharded, in_names, out_names, [z.shape for z in zero_outs],
              [z.dtype for z in zero_outs])
    _STATE["runner"] = runner
    _STATE["nc"] = nc
    return runner


def _prep_core_inputs(x, weight, k):
    """Host-side slicing/layout for core k (i-shard of 144 capsules)."""
    i0, i1 = k * IL, (k + 1) * IL
    xs = np.ascontiguousarray(x[:, :, i0:i1])          # [256, 8, 144]
    ws = np.ascontiguousarray(weight[i0:i1])           # [144, 10, 16, 8]

    # [(i,d), b] i-major rows, then partition-chunked to [128, 9*256]
    xr = xs.transpose(2, 1, 0).reshape(ID, B)
    xr_ch = xr.reshape(NCH, 128, B).transpose(1, 0, 2).reshape(128, NCH * B)
    # [b, (i,d)] i-major cols, b-chunked to [128, 2*1152]
    xid = xs.transpose(0, 2, 1).reshape(B, IL * DI)
    xid_ch = np.concatenate([xid[0:128], xid[128:256]], axis=1)
    # [(i,d), (j,o)] -> chunked [128, 9*160]
    wt = ws.transpose(0, 3, 1, 2).reshape(ID, JO)
    wt_ch = wt.reshape(NCH, 128, JO).transpose(1, 0, 2).reshape(128, NCH * JO)

    return {
        "xr": xr_ch.astype(np.float16),
        "wt": wt_ch.astype(np.float16),
        "xid": xid_ch.astype(np.float16),
        "amat": _STATE["amat"],
        "tsum": _STATE["tsum"],
    }


def _selectors():
    if "amat" not in _STATE:
        # block-diagonal (i,d)->(i,d') d-sum matrix with the 1/B mean folded in
        p = np.arange(128)
        _STATE["amat"] = ((p[:, None] // DI == p[None, :] // DI)
                          .astype(np.float32) / B).astype(np.float16)
        # [128,16] tiled identity: sums the 8 16-row A2A blocks via matmul
        _STATE["tsum"] = np.tile(np.eye(16, dtype=np.float16), (8, 1))


def kernel(x, weight):
    x = np.asarray(x, dtype=np.float32)
    weight = np.asarray(weight, dtype=np.float32)
    _selectors()
    sharded, in_names, out_names, out_shapes, out_dtypes = _get_runner()

    per_core = [_prep_core_inputs(x, weight, k) for k in range(N_CORES)]
    concat_in = [
        np.concatenate([per_core[c][nm] for c in range(N_CORES)], axis=0)
        for nm in in_names
    ]
    concat_zero = [
        np.zeros((N_CORES * s[0],) + tuple(s[1:]), d)
        for s, d in zip(out_shapes, out_dtypes)
    ]
    outs = sharded(*concat_in, *concat_zero)
    v = np.asarray(outs[out_names.index("v_out")])   # [8*16, 320]
    full = np.empty((B, JO), np.float32)
    for k in range(N_CORES):
        vk = v[k * 16:(k + 1) * 16]
        full[k * 16:k * 16 + 16] = vk[:, :JO]
        full[128 + k * 16:128 + k * 16 + 16] = vk[:, JO:]
    return full.reshape(B, NO, DO)


# revision 24
# speedup vs baseline: 1.0346x; 1.0019x over previous
"""Trainium2 Bass kernel for a DGL-style digit-capsule routing layer.

Inputs (full, unsharded):
    x      [256, 8, 1152] f32   -- B, D_IN, N_IN
    weight [1152, 10, 16, 8] f32 -- N_IN, N_OUT, D_OUT, D_IN
Output:
    v      [256, 10, 16] f32

Algorithm (exact refactor of the reference, never materializing u_hat):
    s[b,(j,o)]    = sum_{(i,d)} c[i,j] W[(i,d),(j,o)] x[b,(i,d)]     (matmul over (i,d))
    v             = squash(s)
    b_upd[i,j]    = (1/B) sum_d sum_o W[(i,d),(j,o)] M[(i,d),(j,o)]
      where M[(i,d),(j,o)] = sum_b x[b,(i,d)] v[b,(j,o)]             (matmul over b)

Sharding: input capsules i are split 8 ways (144 per core). Per routing
iteration the only cross-core data is the partial sum of s ([256,160],
carried in fp16): iterations 1-2 use AllReduce; iteration 3 uses
AllToAll + an on-core partition-sum matmul (A2A is ~2x cheaper than
ReduceScatter since the CCE reduce needs two source reads per chunk),
after which each core squashes and emits its own 32-batch-row shard.

Perf notes vs the 71.2us baseline (measured 66.2us, rel err 2.1e-3):
  - final iteration: ReduceScatter (9.9us) -> AllToAll (~6.3us) + one
    tiled-identity matmul that sums the 8 gathered 16-row blocks on the
    idle tensor engine, landing full s for this core's batch slice in
    PSUM fp32 (squash squares it on the scalar engine: one PSUM read).
    Output shrinks to [16, 320] per core; the host re-interleaves.
  - agreement W*M multiplies and o-reductions fused 9->3 ops each; each
    PSUM tile holds three CONSECUTIVE chunks so the fused vector work
    for tile g starts after g*6+6 of the 18 M-matmuls. Groups 0/1 stage
    PSUM->fp16 on the idle scalar engine (all-16-bit mult -> DVE 2x);
    the tail-bound group 2 reads PSUM directly.
  - squash: the 10-op rsqrt Newton chain is now 6 ops via a custom DVE
    instruction (SQUASH_SQRT_NR, registered at build time: fused
    (q*y0)*(1.5-0.5*q*y0^2) Newton polish) plus the pre-registered
    51-ULP reciprocal_approx_fast for 1/(1+q); the squared tile is fp16
    so the square runs in DVE 2x mode. Softmax's 1/z uses
    reciprocal_approx_fast too.
  - Wc computed in 3 fused ops (30 capsule groups each) instead of 9.
  - collective staging is ONE [128,320] fp16 DMA per hop (640B rows:
    two half-payload DMAs would pay the <512B descriptor 2x latency
    penalty); the b_ch bookkeeping copy is emitted off the exp->z->c
    chain.

Measured per-iteration anatomy (middle iteration, steady state):
  AllReduce trigger-to-done 12.3us + in-hop 2.1 + squash 2.5 +
  M/agreement 4.8 + softmax 1.5 + Wc+s-matmul 2.9 + copies/out-hop 3.0
  ~= 29.3us. The three collective rounds are serial by data dependence
  (squash is nonlinear, so each routing iteration's global s-sum feeds
  the next); AllReduce is the cheapest reduce+replicate (A2A+AllGather
  decompositions pay the ~5us hop+trigger overhead twice).
"""

import numpy as np

N_CORES = 8
B = 256
NI, NO, DO, DI = 1152, 10, 16, 8
JO = NO * DO            # 160
IL = NI // N_CORES      # 144 capsules per core
ID = IL * DI            # 1152 (i,d) rows per core
NCH = ID // 128         # 9 partition chunks
BL = B // N_CORES       # 32 output batch rows per core
SQRT_MAGIC = 0x1FBD1DF5  # bits(sqrt(x)) ~= (bits(x)>>1) + MAGIC

_STATE = {}


def _register_dve_ops():
    """Register the fused sqrt-Newton custom DVE op (idempotent).

    SQUASH_SQRT_NR: out = (q*y0) * (c0 - c1*(q*y0*y0)) with Src0=q,
    Src1=y0~rsqrt(q) seed, c0=1.5, c1=0.5 -> one-instruction Newton
    polish producing sqrt(q) to ~0.2%.
    """
    import concourse.dve_ops as dops
    from concourse.dve_ops import DveOp, DveOpSpec
    from concourse.dve_spec import Spec, Src0, Src1, C0, C1, lower

    if "SQUASH_SQRT_NR" in dops._SUB_OPCODE_FOR_NAME:
        for op in dops.OPS:
            if op.name == "SQUASH_SQRT_NR":
                return op

    a = Src0 * Src1
    body = a * (C0 - C1 * (a * Src1))

    def _ref(in0, in1, c0, c1, c2):
        aa = in0 * in1
        return aa * (c0 - c1 * (aa * in1))

    op = DveOp("SQUASH_SQRT_NR", Spec(body=body, reference=_ref),
               subdim=False, uops_sha={})
    dops.OPS.append(op)
    dops.CUSTOM_DVE_SPECS[op.name] = op.spec
    dops._SUB_OPCODE_FOR_NAME[op.name] = (
        dops._CUSTOM_DVE_ROW_BASE + len(dops.OPS) - 1)
    for ver in ("v3", "v4"):
        uops = lower(op.spec, ver=ver)
        op.uops_sha[ver] = DveOpSpec(
            name=op.name, opcode=dops.get_dve_sub_opcode(op.name),
            uops=uops, rd1_en=True).sha(ver)
    return op


def _build(repeat=1):
    """Build the bass program. ``repeat`` > 1 duplicates the full routing
    computation (for slope-based HW timing); the output is unchanged."""
    import concourse.bass as bass
    import concourse.bacc as bacc
    import concourse.mybir as mybir
    import concourse.tile as tile

    dt = mybir.dt
    AF = mybir.ActivationFunctionType
    ALU = mybir.AluOpType

    sq_op = _register_dve_ops()
    nc = bacc.Bacc(None, num_devices=N_CORES)

    # Per-core external inputs (pre-sharded/pre-laid-out on host).
    xr = nc.declare_dram_parameter("xr", [128, NCH * B], dt.float16, isOutput=False)
    wt = nc.declare_dram_parameter("wt", [128, NCH * JO], dt.float16, isOutput=False)
    xid = nc.declare_dram_parameter("xid", [128, 2 * ID], dt.float16, isOutput=False)
    amat = nc.declare_dram_parameter("amat", [128, 128], dt.float16, isOutput=False)
    tsum = nc.declare_dram_parameter("tsum", [128, 16], dt.float16, isOutput=False)
    v_out = nc.declare_dram_parameter("v_out", [16, 2 * JO], dt.float32,
                                      isOutput=True)

    # Internal DRAM bounce buffers for the collectives (per repeat x iter).
    # Collective payloads live as [128, 2*JO]: batch half kb sits in the
    # column range [kb*JO, (kb+1)*JO) so each hop is ONE contiguous DMA.
    cc_in_all, cc_out_all = [], []
    for rep in range(repeat):
        cc_in_all.append(
            [nc.dram_tensor(f"cc_in{rep}_{t}", [128, 2 * JO], dt.float16)
             for t in range(3)])
        # AllReduce outputs need Shared addr space; AllToAll must not
        cc_out_all.append(
            [nc.dram_tensor(f"cc_out{rep}_{t}", [128, 2 * JO], dt.float16,
                            **({"addr_space": "Shared"} if t < 2 else {}))
             for t in range(3)])
    groups = [list(range(N_CORES))]

    with tile.TileContext(nc) as tc:
        with tc.tile_pool(name="const", bufs=1) as cpool, \
             tc.tile_pool(name="work", bufs=2) as wpool, \
             tc.tile_pool(name="sq", bufs=2) as qpool, \
             tc.tile_pool(name="psum_s", bufs=2, space="PSUM") as ps_pool, \
             tc.tile_pool(name="psum_m", bufs=1, space="PSUM") as pm_pool, \
             tc.tile_pool(name="psum_v", bufs=1, space="PSUM") as pv_pool, \
             tc.tile_pool(name="psum_b", bufs=1, space="PSUM") as pb_pool:

            # ---- constant/persistent tiles ----
            xr_t = cpool.tile([128, NCH * B], dt.float16, tag="xr")
            wt_t = cpool.tile([128, NCH * JO], dt.float16, tag="wt")
            xid_t = cpool.tile([128, 2 * ID], dt.float16, tag="xid")
            amat_t = cpool.tile([128, 128], dt.float16, tag="amat")
            tsum_t = cpool.tile([128, 16], dt.float16, tag="tsum")
            b_ch = cpool.tile([128, NCH * NO], dt.float32, tag="bch")
            wc_t = cpool.tile([128, NCH * JO], dt.float16, tag="wc")

            nc.sync.dma_start(xr_t[:], xr[:])
            nc.sync.dma_start(wt_t[:], wt[:])
            nc.sync.dma_start(xid_t[:], xid[:])
            nc.sync.dma_start(amat_t[:], amat[:])
            nc.sync.dma_start(tsum_t[:], tsum[:])

            def squash(sf, p, out_dt, tag, ng=NO, in_psum=False):
                """v = s * sqrt(sq)/(1+sq); rsqrt via sqrt-bits trick
                + reciprocal + one Newton step, all on the vector engine.
                ``ng`` capsule groups of DO columns are squashed at once."""
                w = ng * DO
                # fp16 out: all-16-bit tensor_tensor runs the DVE in 2x mode
                t2 = wpool.tile([p, w], dt.float16, tag=f"ssq{tag}")
                if in_psum:
                    # sf*sf would need two PSUM reads; DVE has one PSUM port
                    nc.scalar.activation(t2[:], sf[:], AF.Square)
                else:
                    nc.vector.tensor_mul(t2[:], sf[:], sf[:])
                sq = qpool.tile([p, ng], dt.float32, tag=f"sq{tag}")
                nc.vector.reduce_sum(
                    out=sq[:], in_=t2[:].rearrange("p (j o) -> p j o", j=ng),
                    axis=mybir.AxisListType.X)
                # fac = sqrt(sq)/(1+sq): bits-trick sqrt seed (one fused
                # shift+add), exact reciprocal -> rsqrt seed, one custom-DVE
                # Newton polish, and a one-instruction approx reciprocal of
                # (1+sq) (51 ULP)
                sb = qpool.tile([p, ng], dt.float32, tag=f"sb{tag}")
                nc.vector.tensor_scalar(
                    sb[:].bitcast(dt.uint32), sq[:].bitcast(dt.uint32),
                    1, None, ALU.logical_shift_right)
                nc.vector.tensor_scalar(
                    sb[:].bitcast(dt.uint32), sb[:].bitcast(dt.uint32),
                    SQRT_MAGIC, None, ALU.add)
                y0 = qpool.tile([p, ng], dt.float32, tag=f"y0{tag}")
                nc.vector.reciprocal(y0[:], sb[:])
                g = qpool.tile([p, ng], dt.float32, tag=f"g{tag}")
                nc.vector._custom_dve(sq_op, out=g[:], in0=sq[:], in1=y0[:],
                                      s0=1.5, s1=0.5)
                r1 = qpool.tile([p, ng], dt.float32, tag=f"r1{tag}")
                nc.vector.tensor_scalar_add(r1[:], sq[:], 1.0)
                rd = qpool.tile([p, ng], dt.float32, tag=f"rd{tag}")
                nc.vector.reciprocal_approx_fast(out=rd[:], in_=r1[:])
                f1 = qpool.tile([p, ng], dt.float32, tag=f"f1{tag}")
                nc.vector.tensor_mul(f1[:], g[:], rd[:])
                vt = wpool.tile([p, w], out_dt, tag=f"v{tag}")
                # two half-width ops: the M-matmuls (which consume batch
                # half kb first) can start ~0.25us earlier
                hg = ng // 2
                for hh in range(2):
                    nc.vector.tensor_tensor(
                        out=vt[:, hh * hg * DO:(hh + 1) * hg * DO]
                            .rearrange("p (j o) -> p j o", j=hg),
                        in0=sf[:, hh * hg * DO:(hh + 1) * hg * DO]
                            .rearrange("p (j o) -> p j o", j=hg),
                        in1=f1[:, hh * hg:(hh + 1) * hg]
                            .unsqueeze(2).to_broadcast((p, hg, DO)),
                        op=ALU.mult)
                return vt

            for rep in range(repeat):
              cc_in = cc_in_all[rep]
              cc_out = cc_out_all[rep]
              for t in range(3):
                # ---- s matmul: psum_s[kb] = sum_ci xr[:,ci,kb]^T @ w ----
                rhs_w = wt_t if t == 0 else wc_t
                st = wpool.tile([128, 2 * JO], dt.float16, tag="s_sb")
                for kb in range(2):
                    ps = ps_pool.tile([128, JO], dt.float32, tag="ps")
                    for ci in range(NCH):
                        lhs = xr_t[:, ci * B + kb * 128: ci * B + kb * 128 + 128]
                        rhs = rhs_w[:, ci * JO:(ci + 1) * JO]
                        nc.tensor.matmul(ps[:], lhs, rhs,
                                         start=(ci == 0), stop=(ci == NCH - 1))
                    # PSUM->fp16 convert on the (otherwise idle) scalar engine;
                    # c == 1/10 exactly on iteration 1: fold into the copy
                    # PSUM->fp16 convert on the vector queue: it sits idle
                    # here and wakes ~1us faster than the scalar queue
                    stk = st[:, kb * JO:(kb + 1) * JO]
                    if t == 0:
                        nc.vector.tensor_scalar_mul(stk, ps[:], 0.1)
                    else:
                        nc.vector.tensor_copy(stk, ps[:])
                # ONE 640B-row DMA for both halves: the collective trigger
                # needs both anyway, and 320B-row descriptors pay the <512B
                # 2x latency penalty
                nc.sync.dma_start(cc_in[t][:], st[:])

                if t == 2:
                    # ---- final iter: AllToAll so 16-row block j of this
                    # core's payload lands on core j; the tiled-identity
                    # matmul sums the 8 received blocks -> full s for OUR
                    # 32 batch rows, in PSUM fp32 ----
                    nc.gpsimd.collective_compute(
                        "AllToAll", ALU.bypass, replica_groups=groups,
                        ins=[cc_in[2][:]], outs=[cc_out[2][:]],
                    )
                    aat = wpool.tile([128, 2 * JO], dt.float16, tag="aat")
                    nc.sync.dma_start(aat[:], cc_out[2][:])
                    ps_v = pv_pool.tile([16, 2 * JO], dt.float32, tag="ps_v")
                    nc.tensor.matmul(ps_v[:], tsum_t[:], aat[:],
                                     start=True, stop=True)
                    v3 = squash(ps_v, 16, dt.float32, 3, ng=2 * NO,
                                in_psum=True)
                    nc.sync.dma_start(v_out[:], v3[:])
                    break

                nc.gpsimd.collective_compute(
                    "AllReduce", ALU.add, replica_groups=groups,
                    ins=[cc_in[t][:]], outs=[cc_out[t][:]],
                )

                # ---- squash both batch halves in one chain ----
                sf2 = wpool.tile([128, 2 * JO], dt.float16, tag="s_full")
                nc.sync.dma_start(sf2[:], cc_out[t][:])
                v2t = squash(sf2, 128, dt.float16, "b", ng=2 * NO)

                # ---- agreement matmuls: M[(i,d),(j,o)] = sum_b x v ----
                # PSUM tile g holds chunks 3g..3g+2 so the fused W*M multiply
                # + o-reduce for tile g can start after 1/3 of the matmuls.
                qt = wpool.tile([128, NCH * NO], dt.float16, tag="qt")
                pms = []
                for g in range(3):
                    pmg = pm_pool.tile([128, 3 * JO], dt.float32, tag=f"pm{g}")
                    pms.append(pmg)
                for g in range(3):
                    pmg = pms[g]
                    for cl in range(3):
                        ci = 3 * g + cl
                        pm = pmg[:, cl * JO:(cl + 1) * JO]
                        for kb in range(2):
                            lhs = xid_t[:, kb * ID + ci * 128:
                                        kb * ID + ci * 128 + 128]
                            nc.tensor.matmul(
                                pm, lhs, v2t[:, kb * JO:(kb + 1) * JO],
                                start=(kb == 0), stop=(kb == 1))
                    pt = wpool.tile([128, 3 * JO], dt.float16, tag=f"pt{g}")
                    if g < 2:
                        # stage PSUM->fp16 on the idle scalar engine so the
                        # W*M multiply runs all-16-bit (DVE 2x, no PSUM port)
                        pmh = wpool.tile([128, 3 * JO], dt.float16,
                                         tag=f"pmh{g}")
                        nc.scalar.copy(pmh[:], pmg[:])
                        nc.vector.tensor_mul(
                            pt[:], pmh[:],
                            wt_t[:, 3 * g * JO:(3 * g + 3) * JO])
                    else:
                        # last group is tail-bound: skip the staging hop and
                        # read PSUM directly
                        nc.vector.tensor_mul(
                            pt[:], pmg[:],
                            wt_t[:, 3 * g * JO:(3 * g + 3) * JO])
                    with nc.allow_low_precision("o-sum, fp16"):
                        nc.vector.reduce_sum(
                            out=qt[:, g * 3 * NO:(g + 1) * 3 * NO],
                            in_=pt[:].rearrange("p (j o) -> p j o", j=3 * NO),
                            axis=mybir.AxisListType.X)

                # ---- b_upd: one matmul with constant block-diag A sums d ----
                pb = pb_pool.tile([128, NCH * NO], dt.float32, tag="pb")
                nc.tensor.matmul(pb[:], amat_t[:], qt[:], start=True, stop=True)

                # ---- softmax over j (free dim within each chunk) ----
                e_ch = wpool.tile([128, NCH * NO], dt.float32, tag="ech")
                if t == 0:
                    nc.scalar.activation(e_ch[:], pb[:], AF.Exp)
                else:
                    nc.vector.tensor_add(b_ch[:], b_ch[:], pb[:])
                    nc.scalar.activation(e_ch[:], b_ch[:], AF.Exp)
                z_ch = wpool.tile([128, NCH], dt.float32, tag="zch")
                nc.vector.reduce_sum(
                    out=z_ch[:], in_=e_ch[:].rearrange("p (c j) -> p c j", c=NCH),
                    axis=mybir.AxisListType.X)
                r_ch = wpool.tile([128, NCH], dt.float32, tag="rch")
                nc.vector.reciprocal_approx_fast(out=r_ch[:], in_=z_ch[:])
                c_ch = wpool.tile([128, NCH * NO], dt.float32, tag="cch")
                nc.vector.tensor_tensor(
                    out=c_ch[:].rearrange("p (c j) -> p c j", c=NCH),
                    in0=e_ch[:].rearrange("p (c j) -> p c j", c=NCH),
                    in1=r_ch[:].unsqueeze(2).to_broadcast((128, NCH, NO)),
                    op=ALU.mult)

                # ---- Wc = Wt * c (fp16), 3 chunks per op so the first
                # s-matmul third can start after one op ----
                for g in range(3):
                    nc.vector.tensor_tensor(
                        out=wc_t[:, g * 3 * JO:(g + 1) * 3 * JO]
                            .rearrange("p (j o) -> p j o", j=3 * NO),
                        in0=wt_t[:, g * 3 * JO:(g + 1) * 3 * JO]
                            .rearrange("p (j o) -> p j o", j=3 * NO),
                        in1=c_ch[:, g * 3 * NO:(g + 1) * 3 * NO]
                            .unsqueeze(2).to_broadcast((128, 3 * NO, DO)),
                        op=ALU.mult)

                if t == 0:
                    # b_ch bookkeeping for t=1 -- emitted last so it never
                    # delays the z-reduce/Wc chain (vector queue is in-order)
                    nc.vector.tensor_copy(b_ch[:], pb[:])

    return nc


def _get_runner():
    if "runner" in _STATE:
        return _STATE["runner"]

    import jax
    import numpy as np
    from concourse import bass2jax
    from concourse.bass2jax import (
        _bass_exec_p, install_neuronx_cc_hook, partition_id_tensor)
    from jax.experimental.shard_map import shard_map
    from jax.sharding import Mesh, PartitionSpec
    import concourse.mybir as mybir

    nc = _build()
    if not nc.is_finalized():
        nc.finalize()
    install_neuronx_cc_hook()

    partition_name = nc.partition_id_tensor.name if nc.partition_id_tensor else None
    in_names, out_names, out_avals, zero_outs = [], [], [], []
    for alloc in nc.m.functions[0].allocations:
        if not isinstance(alloc, mybir.MemoryLocationSet):
            continue
        name = alloc.memorylocations[0].name
        if alloc.kind == "ExternalInput":
            if name != partition_name:
                in_names.append(name)
        elif alloc.kind == "ExternalOutput":
            out_names.append(name)
            shape = tuple(alloc.tensor_shape)
            dtype = mybir.dt.np(alloc.dtype)
            out_avals.append(jax.core.ShapedArray(shape, dtype))
            zero_outs.append(np.zeros(shape, dtype))
    n_params = len(in_names)
    n_outs = len(out_avals)
    all_names = in_names + out_names
    if partition_name is not None:
        all_names = all_names + [partition_name]

    def _body(*args):
        operands = list(args)
        if partition_name is not None:
            operands.append(partition_id_tensor())
        outs = _bass_exec_p.bind(
            *operands,
            out_avals=tuple(out_avals),
            in_names=tuple(all_names),
            out_names=tuple(out_names),
            lowering_input_output_aliases=(),
            sim_require_finite=True,
            sim_require_nnan=True,
            nc=nc,
        )
        return tuple(outs)

    devices = jax.devices()[:N_CORES]
    assert len(devices) == N_CORES, f"need {N_CORES} cores, have {len(devices)}"
    mesh = Mesh(np.asarray(devices), ("core",))
    donate = tuple(range(n_params, n_params + n_outs))
    sharded = jax.jit(
        shard_map(_body, mesh=mesh,
                  in_specs=(PartitionSpec("core"),) * (n_params + n_outs),
                  out_specs=(PartitionSpec("core"),) * n_outs,
                  check_rep=False),
        donate_argnums=donate, keep_unused=True)

    runner = (sharded, in_names, out_names, [z.shape for z in zero_outs],
              [z.dtype for z in zero_outs])
    _STATE["runner"] = runner
    _STATE["nc"] = nc
    return runner


def _prep_core_inputs(x, weight, k):
    """Host-side slicing/layout for core k (i-shard of 144 capsules)."""
    i0, i1 = k * IL, (k + 1) * IL
    xs = np.ascontiguousarray(x[:, :, i0:i1])          # [256, 8, 144]
    ws = np.ascontiguousarray(weight[i0:i1])           # [144, 10, 16, 8]

    # [(i,d), b] i-major rows, then partition-chunked to [128, 9*256]
    xr = xs.transpose(2, 1, 0).reshape(ID, B)
    xr_ch = xr.reshape(NCH, 128, B).transpose(1, 0, 2).reshape(128, NCH * B)
    # [b, (i,d)] i-major cols, b-chunked to [128, 2*1152]
    xid = xs.transpose(0, 2, 1).reshape(B, IL * DI)
    xid_ch = np.concatenate([xid[0:128], xid[128:256]], axis=1)
    # [(i,d), (j,o)] -> chunked [128, 9*160]
    wt = ws.transpose(0, 3, 1, 2).reshape(ID, JO)
    wt_ch = wt.reshape(NCH, 128, JO).transpose(1, 0, 2).reshape(128, NCH * JO)

    return {
        "xr": xr_ch.astype(np.float16),
        "wt": wt_ch.astype(np.float16),
        "xid": xid_ch.astype(np.float16),
        "amat": _STATE["amat"],
        "tsum": _STATE["tsum"],
    }


def _selectors():
    if "amat" not in _STATE:
        # block-diagonal (i,d)->(i,d') d-sum matrix with the 1/B mean folded in
        p = np.arange(128)
        _STATE["amat"] = ((p[:, None] // DI == p[None, :] // DI)
                          .astype(np.float32) / B).astype(np.float16)
        # [128,16] tiled identity: sums the 8 16-row A2A blocks via matmul
        _STATE["tsum"] = np.tile(np.eye(16, dtype=np.float16), (8, 1))


def kernel(x, weight):
    x = np.asarray(x, dtype=np.float32)
    weight = np.asarray(weight, dtype=np.float32)
    _selectors()
    sharded, in_names, out_names, out_shapes, out_dtypes = _get_runner()

    per_core = [_prep_core_inputs(x, weight, k) for k in range(N_CORES)]
    concat_in = [
        np.concatenate([per_core[c][nm] for c in range(N_CORES)], axis=0)
        for nm in in_names
    ]
    concat_zero = [
        np.zeros((N_CORES * s[0],) + tuple(s[1:]), d)
        for s, d in zip(out_shapes, out_dtypes)
    ]
    outs = sharded(*concat_in, *concat_zero)
    v = np.asarray(outs[out_names.index("v_out")])   # [8*16, 320]
    full = np.empty((B, JO), np.float32)
    for k in range(N_CORES):
        vk = v[k * 16:(k + 1) * 16]
        full[k * 16:k * 16 + 16] = vk[:, :JO]
        full[128 + k * 16:128 + k * 16 + 16] = vk[:, JO:]
    return full.reshape(B, NO, DO)


# revision 29
# speedup vs baseline: 1.0368x; 1.0021x over previous
"""Trainium2 Bass kernel for a DGL-style digit-capsule routing layer.

Inputs (full, unsharded):
    x      [256, 8, 1152] f32   -- B, D_IN, N_IN
    weight [1152, 10, 16, 8] f32 -- N_IN, N_OUT, D_OUT, D_IN
Output:
    v      [256, 10, 16] f32

Algorithm (exact refactor of the reference, never materializing u_hat):
    s[b,(j,o)]    = sum_{(i,d)} c[i,j] W[(i,d),(j,o)] x[b,(i,d)]     (matmul over (i,d))
    v             = squash(s)
    b_upd[i,j]    = (1/B) sum_d sum_o W[(i,d),(j,o)] M[(i,d),(j,o)]
      where M[(i,d),(j,o)] = sum_b x[b,(i,d)] v[b,(j,o)]             (matmul over b)

Sharding: input capsules i are split 8 ways (144 per core). Per routing
iteration the only cross-core data is the partial sum of s ([256,160],
carried in fp16): iterations 1-2 use AllReduce; iteration 3 uses
AllToAll + an on-core partition-sum matmul (A2A is ~2x cheaper than
ReduceScatter since the CCE reduce needs two source reads per chunk),
after which each core squashes and emits its own 32-batch-row shard.

Perf notes vs the 71.2us baseline (measured 66.2us, rel err 2.1e-3):
  - final iteration: ReduceScatter (9.9us) -> AllToAll (~6.3us) + one
    tiled-identity matmul that sums the 8 gathered 16-row blocks on the
    idle tensor engine, landing full s for this core's batch slice in
    PSUM fp32 (squash squares it on the scalar engine: one PSUM read).
    Output shrinks to [16, 320] per core; the host re-interleaves.
  - agreement W*M multiplies and o-reductions fused 9->3 ops each; each
    PSUM tile holds three CONSECUTIVE chunks so the fused vector work
    for tile g starts after g*6+6 of the 18 M-matmuls. Groups 0/1 stage
    PSUM->fp16 on the idle scalar engine (all-16-bit mult -> DVE 2x);
    the tail-bound group 2 reads PSUM directly.
  - squash: the 10-op rsqrt Newton chain is now 6 ops via a custom DVE
    instruction (SQUASH_SQRT_NR, registered at build time: fused
    (q*y0)*(1.5-0.5*q*y0^2) Newton polish) plus the pre-registered
    51-ULP reciprocal_approx_fast for 1/(1+q); the squared tile is fp16
    so the square runs in DVE 2x mode. Softmax's 1/z uses
    reciprocal_approx_fast too.
  - Wc computed in 3 fused ops (30 capsule groups each) instead of 9.
  - collective staging is ONE [128,320] fp16 DMA per hop (640B rows:
    two half-payload DMAs would pay the <512B descriptor 2x latency
    penalty); the routing logits accumulate in PSUM across the two
    updates (t=0 opens the matmul accumulation group, t=1 closes it),
    so no logit copy/add ever touches the exp chain.

Measured per-iteration anatomy (middle iteration, steady state):
  AllReduce trigger-to-done 12.3us + in-hop 2.1 + squash 2.5 +
  M/agreement 4.8 + softmax 1.5 + Wc+s-matmul 2.9 + copies/out-hop 3.0
  ~= 29.3us. The three collective rounds are serial by data dependence
  (squash is nonlinear, so each routing iteration's global s-sum feeds
  the next); AllReduce is the cheapest reduce+replicate (A2A+AllGather
  decompositions pay the ~5us hop+trigger overhead twice).
"""

import numpy as np

N_CORES = 8
B = 256
NI, NO, DO, DI = 1152, 10, 16, 8
JO = NO * DO            # 160
IL = NI // N_CORES      # 144 capsules per core
ID = IL * DI            # 1152 (i,d) rows per core
NCH = ID // 128         # 9 partition chunks
BL = B // N_CORES       # 32 output batch rows per core
SQRT_MAGIC = 0x1FBD1DF5  # bits(sqrt(x)) ~= (bits(x)>>1) + MAGIC

_STATE = {}


def _register_dve_ops():
    """Register the fused sqrt-Newton custom DVE op (idempotent).

    SQUASH_SQRT_NR: out = (q*y0) * (c0 - c1*(q*y0*y0)) with Src0=q,
    Src1=y0~rsqrt(q) seed, c0=1.5, c1=0.5 -> one-instruction Newton
    polish producing sqrt(q) to ~0.2%.
    """
    import concourse.dve_ops as dops
    from concourse.dve_ops import DveOp, DveOpSpec
    from concourse.dve_spec import Spec, Src0, Src1, C0, C1, lower

    if "SQUASH_SQRT_NR" in dops._SUB_OPCODE_FOR_NAME:
        for op in dops.OPS:
            if op.name == "SQUASH_SQRT_NR":
                return op

    a = Src0 * Src1
    body = a * (C0 - C1 * (a * Src1))

    def _ref(in0, in1, c0, c1, c2):
        aa = in0 * in1
        return aa * (c0 - c1 * (aa * in1))

    op = DveOp("SQUASH_SQRT_NR", Spec(body=body, reference=_ref),
               subdim=False, uops_sha={})
    dops.OPS.append(op)
    dops.CUSTOM_DVE_SPECS[op.name] = op.spec
    dops._SUB_OPCODE_FOR_NAME[op.name] = (
        dops._CUSTOM_DVE_ROW_BASE + len(dops.OPS) - 1)
    for ver in ("v3", "v4"):
        uops = lower(op.spec, ver=ver)
        op.uops_sha[ver] = DveOpSpec(
            name=op.name, opcode=dops.get_dve_sub_opcode(op.name),
            uops=uops, rd1_en=True).sha(ver)
    return op


def _build(repeat=1):
    """Build the bass program. ``repeat`` > 1 duplicates the full routing
    computation (for slope-based HW timing); the output is unchanged."""
    import concourse.bass as bass
    import concourse.bacc as bacc
    import concourse.mybir as mybir
    import concourse.tile as tile

    dt = mybir.dt
    AF = mybir.ActivationFunctionType
    ALU = mybir.AluOpType

    sq_op = _register_dve_ops()
    nc = bacc.Bacc(None, num_devices=N_CORES)

    # Per-core external inputs (pre-sharded/pre-laid-out on host).
    xr = nc.declare_dram_parameter("xr", [128, NCH * B], dt.float16, isOutput=False)
    wt = nc.declare_dram_parameter("wt", [128, NCH * JO], dt.float16, isOutput=False)
    xid = nc.declare_dram_parameter("xid", [128, 2 * ID], dt.float16, isOutput=False)
    amat = nc.declare_dram_parameter("amat", [128, 128], dt.float16, isOutput=False)
    tsum = nc.declare_dram_parameter("tsum", [128, 16], dt.float16, isOutput=False)
    v_out = nc.declare_dram_parameter("v_out", [16, 2 * JO], dt.float32,
                                      isOutput=True)

    # Internal DRAM bounce buffers for the collectives (per repeat x iter).
    # Collective payloads live as [128, 2*JO]: batch half kb sits in the
    # column range [kb*JO, (kb+1)*JO) so each hop is ONE contiguous DMA.
    cc_in_all, cc_out_all = [], []
    for rep in range(repeat):
        cc_in_all.append(
            [nc.dram_tensor(f"cc_in{rep}_{t}", [128, 2 * JO], dt.float16)
             for t in range(3)])
        # AllReduce outputs need Shared addr space; AllToAll must not
        cc_out_all.append(
            [nc.dram_tensor(f"cc_out{rep}_{t}", [128, 2 * JO], dt.float16,
                            **({"addr_space": "Shared"} if t < 2 else {}))
             for t in range(3)])
    groups = [list(range(N_CORES))]

    with tile.TileContext(nc) as tc:
        with tc.tile_pool(name="const", bufs=1) as cpool, \
             tc.tile_pool(name="work", bufs=2) as wpool, \
             tc.tile_pool(name="sq", bufs=2) as qpool, \
             tc.tile_pool(name="psum_s", bufs=2, space="PSUM") as ps_pool, \
             tc.tile_pool(name="psum_m", bufs=1, space="PSUM") as pm_pool, \
             tc.tile_pool(name="psum_v", bufs=1, space="PSUM") as pv_pool, \
             tc.tile_pool(name="psum_b", bufs=1, space="PSUM") as pb_pool:

            # ---- constant/persistent tiles ----
            xr_t = cpool.tile([128, NCH * B], dt.float16, tag="xr")
            wt_t = cpool.tile([128, NCH * JO], dt.float16, tag="wt")
            xid_t = cpool.tile([128, 2 * ID], dt.float16, tag="xid")
            amat_t = cpool.tile([128, 128], dt.float16, tag="amat")
            tsum_t = cpool.tile([128, 16], dt.float16, tag="tsum")
            wc_t = cpool.tile([128, NCH * JO], dt.float16, tag="wc")

            nc.sync.dma_start(xr_t[:], xr[:])
            nc.sync.dma_start(wt_t[:], wt[:])
            nc.sync.dma_start(xid_t[:], xid[:])
            nc.sync.dma_start(amat_t[:], amat[:])
            nc.sync.dma_start(tsum_t[:], tsum[:])

            def squash(sf, p, out_dt, tag, ng=NO, in_psum=False):
                """v = s * sqrt(sq)/(1+sq); rsqrt via sqrt-bits trick
                + reciprocal + one Newton step, all on the vector engine.
                ``ng`` capsule groups of DO columns are squashed at once."""
                w = ng * DO
                # fp16 out: all-16-bit tensor_tensor runs the DVE in 2x mode
                t2 = wpool.tile([p, w], dt.float16, tag=f"ssq{tag}")
                if in_psum:
                    # sf*sf would need two PSUM reads; DVE has one PSUM port
                    nc.scalar.activation(t2[:], sf[:], AF.Square)
                else:
                    nc.vector.tensor_mul(t2[:], sf[:], sf[:])
                sq = qpool.tile([p, ng], dt.float32, tag=f"sq{tag}")
                nc.vector.reduce_sum(
                    out=sq[:], in_=t2[:].rearrange("p (j o) -> p j o", j=ng),
                    axis=mybir.AxisListType.X)
                # fac = sqrt(sq)/(1+sq): bits-trick sqrt seed (one fused
                # shift+add), exact reciprocal -> rsqrt seed, one custom-DVE
                # Newton polish, and a one-instruction approx reciprocal of
                # (1+sq) (51 ULP)
                sb = qpool.tile([p, ng], dt.float32, tag=f"sb{tag}")
                nc.vector.tensor_scalar(
                    sb[:].bitcast(dt.uint32), sq[:].bitcast(dt.uint32),
                    1, None, ALU.logical_shift_right)
                nc.vector.tensor_scalar(
                    sb[:].bitcast(dt.uint32), sb[:].bitcast(dt.uint32),
                    SQRT_MAGIC, None, ALU.add)
                y0 = qpool.tile([p, ng], dt.float32, tag=f"y0{tag}")
                nc.vector.reciprocal(y0[:], sb[:])
                g = qpool.tile([p, ng], dt.float32, tag=f"g{tag}")
                nc.vector._custom_dve(sq_op, out=g[:], in0=sq[:], in1=y0[:],
                                      s0=1.5, s1=0.5)
                r1 = qpool.tile([p, ng], dt.float32, tag=f"r1{tag}")
                nc.vector.tensor_scalar_add(r1[:], sq[:], 1.0)
                rd = qpool.tile([p, ng], dt.float32, tag=f"rd{tag}")
                nc.vector.reciprocal_approx_fast(out=rd[:], in_=r1[:])
                f1 = qpool.tile([p, ng], dt.float32, tag=f"f1{tag}")
                nc.vector.tensor_mul(f1[:], g[:], rd[:])
                vt = wpool.tile([p, w], out_dt, tag=f"v{tag}")
                nc.vector.tensor_tensor(
                    out=vt[:].rearrange("p (j o) -> p j o", j=ng),
                    in0=sf[:].rearrange("p (j o) -> p j o", j=ng),
                    in1=f1[:].unsqueeze(2).to_broadcast((p, ng, DO)),
                    op=ALU.mult)
                return vt

            for rep in range(repeat):
              cc_in = cc_in_all[rep]
              cc_out = cc_out_all[rep]
              for t in range(3):
                # ---- s matmul: psum_s[kb] = sum_ci xr[:,ci,kb]^T @ w ----
                rhs_w = wt_t if t == 0 else wc_t
                st = wpool.tile([128, 2 * JO], dt.float16, tag="s_sb")
                for kb in range(2):
                    ps = ps_pool.tile([128, JO], dt.float32, tag="ps")
                    for ci in range(NCH):
                        lhs = xr_t[:, ci * B + kb * 128: ci * B + kb * 128 + 128]
                        rhs = rhs_w[:, ci * JO:(ci + 1) * JO]
                        nc.tensor.matmul(ps[:], lhs, rhs,
                                         start=(ci == 0), stop=(ci == NCH - 1))
                    # PSUM->fp16 convert on the (otherwise idle) scalar engine;
                    # c == 1/10 exactly on iteration 1: fold into the copy
                    # PSUM->fp16 convert on the (otherwise idle) scalar engine;
                    # c == 1/10 exactly on iteration 1: fold into the copy
                    stk = st[:, kb * JO:(kb + 1) * JO]
                    if t == 0:
                        nc.scalar.mul(stk, ps[:], 0.1)
                    else:
                        nc.scalar.copy(stk, ps[:])
                # ONE 640B-row DMA for both halves: the collective trigger
                # needs both anyway, and 320B-row descriptors pay the <512B
                # 2x latency penalty
                nc.sync.dma_start(cc_in[t][:], st[:])

                if t == 2:
                    # ---- final iter: AllToAll so 16-row block j of this
                    # core's payload lands on core j; the tiled-identity
                    # matmul sums the 8 received blocks -> full s for OUR
                    # 32 batch rows, in PSUM fp32 ----
                    nc.gpsimd.collective_compute(
                        "AllToAll", ALU.bypass, replica_groups=groups,
                        ins=[cc_in[2][:]], outs=[cc_out[2][:]],
                    )
                    aat = wpool.tile([128, 2 * JO], dt.float16, tag="aat")
                    nc.sync.dma_start(aat[:], cc_out[2][:])
                    ps_v = pv_pool.tile([16, 2 * JO], dt.float32, tag="ps_v")
                    nc.tensor.matmul(ps_v[:], tsum_t[:], aat[:],
                                     start=True, stop=True)
                    v3 = squash(ps_v, 16, dt.float32, 3, ng=2 * NO,
                                in_psum=True)
                    nc.sync.dma_start(v_out[:], v3[:])
                    break

                nc.gpsimd.collective_compute(
                    "AllReduce", ALU.add, replica_groups=groups,
                    ins=[cc_in[t][:]], outs=[cc_out[t][:]],
                )

                # ---- squash both batch halves in one chain ----
                sf2 = wpool.tile([128, 2 * JO], dt.float16, tag="s_full")
                nc.sync.dma_start(sf2[:], cc_out[t][:])
                v2t = squash(sf2, 128, dt.float16, "b", ng=2 * NO)

                # ---- agreement matmuls: M[(i,d),(j,o)] = sum_b x v ----
                # PSUM tile g holds chunks 3g..3g+2 so the fused W*M multiply
                # + o-reduce for tile g can start after 1/3 of the matmuls.
                qt = wpool.tile([128, NCH * NO], dt.float16, tag="qt")
                pms = []
                for g in range(3):
                    pmg = pm_pool.tile([128, 3 * JO], dt.float32, tag=f"pm{g}")
                    pms.append(pmg)
                for g in range(3):
                    pmg = pms[g]
                    for cl in range(3):
                        ci = 3 * g + cl
                        pm = pmg[:, cl * JO:(cl + 1) * JO]
                        for kb in range(2):
                            lhs = xid_t[:, kb * ID + ci * 128:
                                        kb * ID + ci * 128 + 128]
                            nc.tensor.matmul(
                                pm, lhs, v2t[:, kb * JO:(kb + 1) * JO],
                                start=(kb == 0), stop=(kb == 1))
                    pt = wpool.tile([128, 3 * JO], dt.float16, tag=f"pt{g}")
                    if g < 2:
                        # stage PSUM->fp16 on the idle scalar engine so the
                        # W*M multiply runs all-16-bit (DVE 2x, no PSUM port)
                        pmh = wpool.tile([128, 3 * JO], dt.float16,
                                         tag=f"pmh{g}")
                        nc.scalar.copy(pmh[:], pmg[:])
                        nc.vector.tensor_mul(
                            pt[:], pmh[:],
                            wt_t[:, 3 * g * JO:(3 * g + 3) * JO])
                    else:
                        # last group is tail-bound: skip the staging hop and
                        # read PSUM directly
                        nc.vector.tensor_mul(
                            pt[:], pmg[:],
                            wt_t[:, 3 * g * JO:(3 * g + 3) * JO])
                    with nc.allow_low_precision("o-sum, fp16"):
                        nc.vector.reduce_sum(
                            out=qt[:, g * 3 * NO:(g + 1) * 3 * NO],
                            in_=pt[:].rearrange("p (j o) -> p j o", j=3 * NO),
                            axis=mybir.AxisListType.X)

                # ---- b_upd: one matmul with constant block-diag A sums d.
                # The logits accumulate IN PSUM across the two updates:
                # t=0 opens the accumulation group, t=1 adds onto it, so no
                # b_ch bookkeeping copy/add ever touches the exp chain ----
                pb = pb_pool.tile([128, NCH * NO], dt.float32, tag="pb")
                nc.tensor.matmul(pb[:], amat_t[:], qt[:],
                                 start=(t == 0), stop=(t == 1),
                                 skip_group_check=True)

                # ---- softmax over j (free dim within each chunk) ----
                e_ch = wpool.tile([128, NCH * NO], dt.float32, tag="ech")
                nc.scalar.activation(e_ch[:], pb[:], AF.Exp)
                z_ch = wpool.tile([128, NCH], dt.float32, tag="zch")
                nc.vector.reduce_sum(
                    out=z_ch[:], in_=e_ch[:].rearrange("p (c j) -> p c j", c=NCH),
                    axis=mybir.AxisListType.X)
                r_ch = wpool.tile([128, NCH], dt.float32, tag="rch")
                nc.vector.reciprocal_approx_fast(out=r_ch[:], in_=z_ch[:])
                c_ch = wpool.tile([128, NCH * NO], dt.float32, tag="cch")
                nc.vector.tensor_tensor(
                    out=c_ch[:].rearrange("p (c j) -> p c j", c=NCH),
                    in0=e_ch[:].rearrange("p (c j) -> p c j", c=NCH),
                    in1=r_ch[:].unsqueeze(2).to_broadcast((128, NCH, NO)),
                    op=ALU.mult)

                # ---- Wc = Wt * c (fp16), 3 chunks per op so the first
                # s-matmul third can start after one op ----
                for g in range(3):
                    nc.vector.tensor_tensor(
                        out=wc_t[:, g * 3 * JO:(g + 1) * 3 * JO]
                            .rearrange("p (j o) -> p j o", j=3 * NO),
                        in0=wt_t[:, g * 3 * JO:(g + 1) * 3 * JO]
                            .rearrange("p (j o) -> p j o", j=3 * NO),
                        in1=c_ch[:, g * 3 * NO:(g + 1) * 3 * NO]
                            .unsqueeze(2).to_broadcast((128, 3 * NO, DO)),
                        op=ALU.mult)

    return nc


def _get_runner():
    if "runner" in _STATE:
        return _STATE["runner"]

    import jax
    import numpy as np
    from concourse import bass2jax
    from concourse.bass2jax import (
        _bass_exec_p, install_neuronx_cc_hook, partition_id_tensor)
    from jax.experimental.shard_map import shard_map
    from jax.sharding import Mesh, PartitionSpec
    import concourse.mybir as mybir

    nc = _build()
    if not nc.is_finalized():
        nc.finalize()
    install_neuronx_cc_hook()

    partition_name = nc.partition_id_tensor.name if nc.partition_id_tensor else None
    in_names, out_names, out_avals, zero_outs = [], [], [], []
    for alloc in nc.m.functions[0].allocations:
        if not isinstance(alloc, mybir.MemoryLocationSet):
            continue
        name = alloc.memorylocations[0].name
        if alloc.kind == "ExternalInput":
            if name != partition_name:
                in_names.append(name)
        elif alloc.kind == "ExternalOutput":
            out_names.append(name)
            shape = tuple(alloc.tensor_shape)
            dtype = mybir.dt.np(alloc.dtype)
            out_avals.append(jax.core.ShapedArray(shape, dtype))
            zero_outs.append(np.zeros(shape, dtype))
    n_params = len(in_names)
    n_outs = len(out_avals)
    all_names = in_names + out_names
    if partition_name is not None:
        all_names = all_names + [partition_name]

    def _body(*args):
        operands = list(args)
        if partition_name is not None:
            operands.append(partition_id_tensor())
        outs = _bass_exec_p.bind(
            *operands,
            out_avals=tuple(out_avals),
            in_names=tuple(all_names),
            out_names=tuple(out_names),
            lowering_input_output_aliases=(),
            sim_require_finite=True,
            sim_require_nnan=True,
            nc=nc,
        )
        return tuple(outs)

    devices = jax.devices()[:N_CORES]
    assert len(devices) == N_CORES, f"need {N_CORES} cores, have {len(devices)}"
    mesh = Mesh(np.asarray(devices), ("core",))
    donate = tuple(range(n_params, n_params + n_outs))
    sharded = jax.jit(
        shard_map(_body, mesh=mesh,
                  in_specs=(PartitionSpec("core"),) * (n_params + n_outs),
                  out_specs=(PartitionSpec("core"),) * n_outs,
                  check_rep=False),
        donate_argnums=donate, keep_unused=True)

    runner = (sharded, in_names, out_names, [z.shape for z in zero_outs],
              [z.dtype for z in zero_outs])
    _STATE["runner"] = runner
    _STATE["nc"] = nc
    return runner


def _prep_core_inputs(x, weight, k):
    """Host-side slicing/layout for core k (i-shard of 144 capsules)."""
    i0, i1 = k * IL, (k + 1) * IL
    xs = np.ascontiguousarray(x[:, :, i0:i1])          # [256, 8, 144]
    ws = np.ascontiguousarray(weight[i0:i1])           # [144, 10, 16, 8]

    # [(i,d), b] i-major rows, then partition-chunked to [128, 9*256]
    xr = xs.transpose(2, 1, 0).reshape(ID, B)
    xr_ch = xr.reshape(NCH, 128, B).transpose(1, 0, 2).reshape(128, NCH * B)
    # [b, (i,d)] i-major cols, b-chunked to [128, 2*1152]
    xid = xs.transpose(0, 2, 1).reshape(B, IL * DI)
    xid_ch = np.concatenate([xid[0:128], xid[128:256]], axis=1)
    # [(i,d), (j,o)] -> chunked [128, 9*160]
    wt = ws.transpose(0, 3, 1, 2).reshape(ID, JO)
    wt_ch = wt.reshape(NCH, 128, JO).transpose(1, 0, 2).reshape(128, NCH * JO)

    return {
        "xr": xr_ch.astype(np.float16),
        "wt": wt_ch.astype(np.float16),
        "xid": xid_ch.astype(np.float16),
        "amat": _STATE["amat"],
        "tsum": _STATE["tsum"],
    }


def _selectors():
    if "amat" not in _STATE:
        # block-diagonal (i,d)->(i,d') d-sum matrix with the 1/B mean folded in
        p = np.arange(128)
        _STATE["amat"] = ((p[:, None] // DI == p[None, :] // DI)
                          .astype(np.float32) / B).astype(np.float16)
        # [128,16] tiled identity: sums the 8 16-row A2A blocks via matmul
        _STATE["tsum"] = np.tile(np.eye(16, dtype=np.float16), (8, 1))


def kernel(x, weight):
    x = np.asarray(x, dtype=np.float32)
    weight = np.asarray(weight, dtype=np.float32)
    _selectors()
    sharded, in_names, out_names, out_shapes, out_dtypes = _get_runner()

    per_core = [_prep_core_inputs(x, weight, k) for k in range(N_CORES)]
    concat_in = [
        np.concatenate([per_core[c][nm] for c in range(N_CORES)], axis=0)
        for nm in in_names
    ]
    concat_zero = [
        np.zeros((N_CORES * s[0],) + tuple(s[1:]), d)
        for s, d in zip(out_shapes, out_dtypes)
    ]
    outs = sharded(*concat_in, *concat_zero)
    v = np.asarray(outs[out_names.index("v_out")])   # [8*16, 320]
    full = np.empty((B, JO), np.float32)
    for k in range(N_CORES):
        vk = v[k * 16:(k + 1) * 16]
        full[k * 16:k * 16 + 16] = vk[:, :JO]
        full[128 + k * 16:128 + k * 16 + 16] = vk[:, JO:]
    return full.reshape(B, NO, DO)
